# revision 2
# baseline (speedup 1.0000x reference)
"""Trainium2 Bass kernel: NeuralNearestNeighbors continuous-KNN weight volumes.

Reference computation (per row of D.reshape(b*m, o), K=8 rounds):
    logits = D / exp(log_temp)
    for k in range(K):
        w_k = log_softmax(logits);  out_k = exp(w_k)
        logits = logits + log1mexp(w_k)          # log(1 - p_k)
    W = stack(out_k, axis=-1)                     # (b, m, o, K)

Exp-space identity: with F_k = softmax(logits_k),
    F_{k+1} = (F_k - F_k^2) / (1 - sum_o F_k^2)
On device we keep an unnormalized (sign-flipped) state G and per-row scalar
g = 1/sum(G) with F = G * g:
    G_0 = exp(D/T)          a_0 = sum(G_0)          g_0 = 1/a_0
    G_{k+1} = (F_k - 1)*F_k a_{k+1} = sum(G_{k+1})  g_{k+1} = 1/a_{k+1}
(signs cancel in F = G*g).

Schedule (v2, round-major): the k-loop is OUTER. Each round k computes
F_k for all 16 row-tiles into one contiguous [P, TILES, O] buffer which is
DMA'd to DRAM as one 4 MB transfer into a k-major output layout
w[K, RPC, O]; the host interleaves K back to last axis during gather
(cheap blocked transpose). Benefits over tile-major:
  - dependent ops (pass1 -> pass2 -> recip -> next pass1) are 16
    instructions apart, so both engines pipeline with no stalls;
  - every engine op is contiguous in SBUF (no 32 B-strided access, which
    cost ~2-3x on ACT writes and DVE reads in the tile-major version);
  - output DMA is 8 x 4 MB (near peak HBM efficiency).

Sharding: purely rowwise data-parallel over b*m = 16384 rows; 2048 rows
per core across 8 cores; log_temp replicated.
"""

import numpy as np

B, M, O = 16, 1024, 512
K = 8
N_CORES = 8
ROWS = B * M                     # 16384
RPC = ROWS // N_CORES            # 2048 rows per core
P = 128
TILES = RPC // P                 # 16 row-tiles per core
IN_DMA_GROUP = 4                 # row-tiles per input DMA (1 MiB transfers)

_cached = None


def _build(variant="v2"):
    """Build and compile the Bass module (one SPMD program for all cores)."""
    from contextlib import ExitStack

    import concourse.bacc as bacc
    import concourse.tile as tile
    from concourse import mybir

    f32 = mybir.dt.float32
    Alu = mybir.AluOpType
    Act = mybir.ActivationFunctionType

    nc = bacc.Bacc(
        "TRN2",
        target_bir_lowering=False,
        debug=False,
        enable_asserts=False,
        num_devices=N_CORES,
    )
    d = nc.dram_tensor("d", [RPC, O], f32, kind="ExternalInput").ap()
    lt = nc.dram_tensor("log_temp", [1, 1], f32, kind="ExternalInput").ap()
    w = nc.dram_tensor("w", [K, RPC, O], f32, kind="ExternalOutput").ap()

    with tile.TileContext(nc) as tc, ExitStack() as ctx:
        singles = ctx.enter_context(tc.tile_pool(name="singles", bufs=1))
        slab_pool = ctx.enter_context(tc.tile_pool(name="slab", bufs=1))
        out_pool = ctx.enter_context(tc.tile_pool(name="out", bufs=3))
        small = ctx.enter_context(tc.tile_pool(name="small", bufs=72))

        # log_temp -> 1/T = exp(-log_temp), replicated to all 128 partitions.
        lt_sb = singles.tile([P, 1], f32)
        nc.sync.dma_start(out=lt_sb[:, :], in_=lt.to_broadcast((P, 1)))
        invt = singles.tile([P, 1], f32)
        nc.scalar.activation(invt[:, :], lt_sb[:, :], Act.Exp, scale=-1.0)

        din = d.rearrange("(t p) o -> p t o", p=P)
        wv = w.rearrange("k (t p) o -> k p t o", p=P)

        # Whole per-core input slab lives in SBUF (32 KB/partition); it is
        # overwritten in place by exp() and each round's G update.
        slab = slab_pool.tile([P, TILES, O], f32)
        for gstart in range(0, TILES, IN_DMA_GROUP):
            # SWDGE path: keeps the HWDGE rings free for output writes.
            nc.gpsimd.dma_start(
                out=slab[:, gstart : gstart + IN_DMA_GROUP, :],
                in_=din[:, gstart : gstart + IN_DMA_GROUP, :],
            )

        # Round 0 prologue: G_0 = exp(D * 1/T), g_0 = 1/rowsum.
        gam = []
        for t in range(TILES):
            acc = small.tile([P, 1], f32)
            g = small.tile([P, 1], f32)
            nc.scalar.activation(
                slab[:, t, :], slab[:, t, :], Act.Exp,
                scale=invt[:, :], accum_out=acc[:, :],
            )
            nc.vector.reciprocal(g[:, :], acc[:, :])
            gam.append(g)

        for k in range(K):
            obuf = out_pool.tile([P, TILES, O], f32)
            for t in range(TILES):
                f_t = obuf[:, t, :]
                g_t = slab[:, t, :]
                # pass1 (ACT): F_k = G * g
                nc.scalar.mul(f_t, g_t, gam[t][:, :])
                if k == K - 1:
                    continue
                # pass2 (DVE): G' = (F - 1) * F, a' = sum(G')
                acc = small.tile([P, 1], f32)
                nc.vector.scalar_tensor_tensor(
                    out=g_t,
                    in0=f_t,
                    scalar=1.0,
                    in1=f_t,
                    op0=Alu.subtract,
                    op1=Alu.mult,
                    accum_out=acc[:, :],
                )
                g = small.tile([P, 1], f32)
                nc.vector.reciprocal(g[:, :], acc[:, :])
                gam[t] = g
            # One 4 MB DMA per round into the k-major layout.
            nc.sync.dma_start(out=wv[k], in_=obuf[:, :, :])

    nc.compile()
    return nc


VARIANT = "v2"


def _get_nc():
    global _cached
    if _cached is None:
        _cached = _build(variant=VARIANT)
    return _cached


def _make_in_maps(D, log_temp):
    Dr = np.ascontiguousarray(np.asarray(D, dtype=np.float32).reshape(ROWS, O))
    lt = np.asarray(log_temp, dtype=np.float32).reshape(1, 1)
    return [
        {"d": Dr[c * RPC : (c + 1) * RPC], "log_temp": lt}
        for c in range(N_CORES)
    ]


def _gather(results):
    out = np.empty((ROWS, O, K), np.float32)
    for c in range(N_CORES):
        arr = results[c]["w"].reshape(K, RPC, O)
        dst = out[c * RPC : (c + 1) * RPC]
        blk = 256
        for r0 in range(0, RPC, blk):
            dst[r0 : r0 + blk] = arr[:, r0 : r0 + blk, :].transpose(1, 2, 0)
    return out.reshape(B, M, O, K)


def run_spmd(D, log_temp, trace=False, **kwargs):
    """Run on all 8 cores; returns (W, BassKernelResults)."""
    from concourse.bass_utils import run_bass_kernel_spmd

    nc = _get_nc()
    res = run_bass_kernel_spmd(
        nc, _make_in_maps(D, log_temp), list(range(N_CORES)), trace=trace, **kwargs
    )
    return _gather(res.results), res


def kernel(D, log_temp):
    W, _ = run_spmd(D, log_temp)
    return W
